# revision 23
# baseline (speedup 1.0000x reference)
"""GRU kernel for Trainium2 (8 NeuronCores, SPMD).

Problem: B=64, T=512, IN=256, H=1024, OUT=10
  gate_x_all = x @ Wx.T + bx            [B,T,3H]
  scan over T:  gate_h = h @ Wh.T + bh
                r = sig(i_r + h_r); i = sig(i_i + h_i)
                n = tanh(i_n + r * h_n)
                h = n + i * (h - n)
  out = h_last @ Wfc.T + bfc            [B,OUT]

Strategy (v10): data-parallel over batch (8 rows/core), weights replicated.
 - bf16 operands everywhere (incl. the h state), fp32 PSUM accumulation.
 - Phase 1: gate_x precomputed for all T in one large weight-streaming GEMM.
 - Phase 2: the T-step scan, software-pipelined in 2 feature-chunks:
   * gate columns per hidden-quarter j ordered [r0|i0|r1|i1|n] (128-chunks)
     so ONE sigmoid covers a chunk's r and i halves.
   * per-step PSUM: pbn (n-gate) + pb0c0/pb0c1 (r,i per chunk) with separate
     accumulation groups -> chunk-0's sigmoid/tanh chain runs while chunk-1
     matmuls still stream.
   * h update: i=sig(.); v2'=(i-1)*tanh-gate (fused); v3=i*h (GpSimd);
     h' = v3 - v2'.
   * h' -> hT via 2 full-width PE transposes (one per 128-chunk), cast into
     separate even/odd hT tiles; next step's matmul k-rounds run evens first
     so streaming restarts before the second chunk's chain finishes.
"""

import os
import sys

import numpy as np

for p in ("/root/.axon_site", "/root/.axon_site/_ro/trn_rl_repo",
          "/root/.axon_site/_ro/pypackages", "/opt/trn_rl_repo"):
    if p not in sys.path:
        sys.path.append(p)

B, T, IN, H, OUT = 64, 512, 256, 1024, 10
H3 = 3 * H
NCORES = 8
BS = B // NCORES      # batch per core (8)
Q = H // 4            # hidden quarter (256)
GF = 3 * Q            # per-group gate columns (768): [r0 i0 r1 i1 | n]
C = 128               # feature chunk within a quarter
P4 = 3 * 32 + BS      # 104: partitions covering all 4 batch groups

_cache = {}


def _build_program():
    import concourse.bass as bass
    import concourse.bacc as bacc
    import concourse.mybir as mybir
    from concourse.tile import TileContext
    from concourse.tile_rust import add_dep_helper

    def after(inst, prev, why="order"):
        # pin engine-queue order (sync=False: no semaphore, FIFO suffices)
        if prev is not None:
            add_dep_helper(inst.ins, prev.ins, sync=False, reason=why)
        return inst

    f32 = mybir.dt.float32
    b16 = mybir.dt.bfloat16
    AF = mybir.ActivationFunctionType
    ALU = mybir.AluOpType

    nc = bacc.Bacc(None, target_bir_lowering=False)

    # ---- per-core inputs ----
    xsT_d = nc.dram_tensor("xsT", [2, 128, T * BS], b16, kind="ExternalInput")
    wxg_d = nc.dram_tensor("wxg", [2, 128, H3], b16, kind="ExternalInput")
    biasg_d = nc.dram_tensor("biasg", [1, H3], b16, kind="ExternalInput")
    # Wh^T: r,i interleaved by 128-chunk [8k,128,4j,2c,256(r|i)], n separate
    whri_d = nc.dram_tensor("whri", [8, 128, 4, 2, 2 * C], b16, kind="ExternalInput")
    whn_d = nc.dram_tensor("whn", [8, 128, 4, Q], b16, kind="ExternalInput")
    bhn_d = nc.dram_tensor("bhn", [1, 4, Q], b16, kind="ExternalInput")
    wfcT_d = nc.dram_tensor("wfcT", [8, 128, OUT], b16, kind="ExternalInput")
    bfc_d = nc.dram_tensor("bfc", [1, OUT], f32, kind="ExternalInput")
    ident_d = nc.dram_tensor("identg", [128, 8], b16, kind="ExternalInput")
    identT_d = nc.dram_tensor("identT", [128, 128], f32, kind="ExternalInput")
    out_d = nc.dram_tensor("out", [BS, OUT], f32, kind="ExternalOutput")

    # gate_x for all T: [m=T/16, p=(16t x 8b), 4*GF] in grouped column order
    gx_d = nc.dram_tensor("gx_all", [T // 16, 128, H3], b16)

    t_steps = int(os.environ.get("KERNEL_T", str(T)))
    KORD = (0, 2, 4, 6, 1, 3, 5, 7)   # evens first: hTe ready before hTo

    with TileContext(nc) as tc:
        # ---------- constants ----------
        with tc.tile_pool(name="const", bufs=1) as cpool:
            whri = cpool.tile([128, 8, 4, 2, 2 * C], b16)
            for k in range(8):
                for j in range(4):
                    nc.sync.dma_start(whri[:, k, j, :, :], whri_d[k, :, j, :, :])
            whn = cpool.tile([128, 8, 4, Q], b16)
            for k in range(8):
                for j in range(4):
                    nc.sync.dma_start(whn[:, k, j, :], whn_d[k, :, j, :])
            bhn = cpool.tile([1, 4, Q], b16)
            nc.sync.dma_start(bhn[:], bhn_d[:])
            wfcT = cpool.tile([128, 8, OUT], b16)
            for k in range(8):
                nc.sync.dma_start(wfcT[:, k, :], wfcT_d[k])
            bfc = cpool.tile([1, OUT], f32)
            nc.sync.dma_start(bfc[:], bfc_d[:])
            ones_b = cpool.tile([1, 128], b16)
            nc.gpsimd.memset(ones_b[:], 1.0)
            ones_f = cpool.tile([1, BS], f32)
            nc.gpsimd.memset(ones_f[:], 1.0)
            identg = cpool.tile([128, 8], b16)
            nc.sync.dma_start(identg[:], ident_d[:])
            identT = cpool.tile([128, 128], f32)
            nc.sync.dma_start(identT[:], identT_d[:])
            identTb = cpool.tile([128, 128], b16)
            nc.vector.tensor_copy(identTb[:], identT[:])

            # ---------- phase 1: gate_x precompute ----------
            with (
                tc.tile_pool(name="px", bufs=2) as pxpool,
                tc.tile_pool(name="pxo", bufs=4) as pxopool,
                tc.tile_pool(name="ppre", bufs=3, space="PSUM") as ppre,
            ):
                xsT = pxpool.tile([128, 2, T * BS], b16)
                for k in range(2):
                    for c in range(8):
                        cs = slice(c * 512, (c + 1) * 512)
                        nc.sync.dma_start(xsT[:, k, cs], xsT_d[k, :, cs])
                wxg = pxpool.tile([128, 2, H3], b16)
                for k in range(2):
                    for c in range(6):
                        cs = slice(c * 512, (c + 1) * 512)
                        nc.sync.dma_start(wxg[:, k, cs], wxg_d[k, :, cs])
                biasg = pxpool.tile([1, H3], b16)
                nc.sync.dma_start(biasg[:], biasg_d[:])

                for m in range(32):          # M-tiles: 128 rows = 16 t x 8 b
                    ms = slice(m * 128, (m + 1) * 128)
                    for j in range(4):       # N-chunks of GF=768 (one group)
                        ncs = slice(j * GF, (j + 1) * GF)
                        pp = ppre.tile([128, GF], f32, tag="pp")
                        for k in range(2):
                            for (o, w) in ((0, 512), (512, 256)):
                                nc.tensor.matmul(
                                    pp[:, o:o + w], xsT[:, k, ms],
                                    wxg[:, k, j * GF + o:j * GF + o + w],
                                    start=(k == 0), stop=False)
                        for (o, w) in ((0, 512), (512, 256)):
                            nc.tensor.matmul(
                                pp[:, o:o + w], ones_b[:],
                                biasg[:, j * GF + o:j * GF + o + w],
                                start=False, stop=True)
                        ob = pxopool.tile([128, GF], b16, tag="ob")
                        if j % 2 == 0:
                            nc.vector.tensor_copy(ob[:], pp[:])
                            nc.sync.dma_start(gx_d[m, :, ncs], ob[:])
                        else:
                            nc.scalar.activation(ob[:], pp[:], AF.Copy)
                            nc.scalar.dma_start(gx_d[m, :, ncs], ob[:])

            # ---------- phase 2: the scan ----------
            with (
                tc.tile_pool(name="state", bufs=3) as spool,
                tc.tile_pool(name="hte", bufs=2) as htepool,
                tc.tile_pool(name="hto", bufs=2) as htopool,
                tc.tile_pool(name="gxt", bufs=6) as gxpool,
                tc.tile_pool(name="gw", bufs=3) as gwork,
                tc.tile_pool(name="pbn", bufs=1, space="PSUM") as pbnpool,
                tc.tile_pool(name="pba", bufs=2, space="PSUM") as pbapool,
                tc.tile_pool(name="pbb", bufs=2, space="PSUM") as pbbpool,
                tc.tile_pool(name="ptr", bufs=2, space="PSUM") as ptrpool,
                tc.tile_pool(name="pwm", bufs=1, space="PSUM") as pwmpool,
            ):
                # HAM keep-warm target: junk matmuls during chain waits keep
                # the PE clock at 8/8 (sub-us idle gaps otherwise oscillate it)
                pwarm = pwmpool.tile([128, 512], f32, tag="pw")
                hTe = htepool.tile([128, 4, BS], b16, tag="hTe")
                nc.gpsimd.memset(hTe[:], 0.0)
                hTo = htopool.tile([128, 4, BS], b16, tag="hTo")
                nc.gpsimd.memset(hTo[:], 0.0)
                hprev = spool.tile([128, Q], b16, tag="h")
                nc.gpsimd.memset(hprev[:], 0.0)

                def g(ap, j):
                    return ap[32 * j:32 * j + BS]

                def hT_blk(k):
                    src = hTe if k % 2 == 0 else hTo
                    return src[:, k // 2, :]

                def junk(n):
                    # deps-free PE filler (constant lhsT, constant rhs)
                    for w in range(n):
                        nc.tensor.matmul(pwarm[0:BS, :], identg[:, 0:BS],
                                         whri[:, w % 8, 0, :, :],
                                         start=True, stop=True)

                # previous step's hnew (chunk-1 transpose deferred into the
                # next step's evens block) + its last DVE chain op
                pend = [None, None]   # (hnew_tile, hy_c1_inst)

                for t in range(t_steps):
                    m, tt = t // 16, t % 16
                    gxt = gxpool.tile([128, GF], b16, tag="gxt")
                    for j in range(4):
                        nc.sync.dma_start(
                            gxt[32 * j:32 * j + BS, :],
                            gx_d[m, tt * BS:(tt + 1) * BS,
                                 j * GF:(j + 1) * GF])

                    pbn = pbnpool.tile([128, Q], f32, tag="pbn")
                    pb0a = pbapool.tile([128, 2 * C], f32, tag="pa")
                    pb0b = pbbpool.tile([128, 2 * C], f32, tag="pb")
                    pb0 = [pb0a, pb0b]
                    hnew = spool.tile([128, Q], b16, tag="h")

                    def inject(c):
                        for j in range(4):
                            nc.tensor.matmul(
                                g(pb0[c], j), g(identg, j),
                                gxt[32 * j:32 * j + BS, 2 * C * c:2 * C * (c + 1)],
                                start=True, stop=False,
                                tile_position=(32 * j, 32 * j))

                    def rounds(c, ks):
                        for k in ks:
                            for j in range(4):
                                nc.tensor.matmul(g(pb0[c], j), hT_blk(k),
                                                 whri[:, k, j, c, :],
                                                 start=False, stop=(k == 7),
                                                 tile_position=(0, 32 * j))

                    def nrounds(ks):
                        for k in ks:
                            for j in range(4):
                                nc.tensor.matmul(g(pbn, j), hT_blk(k),
                                                 whn[:, k, j, :],
                                                 start=False, stop=(k == 7),
                                                 tile_position=(0, 32 * j))

                    # ---- evens block: needs only hTe (cast_c0 of t-1) ----
                    junk(2)
                    inject(0)
                    rounds(0, KORD[:4])
                    for j in range(4):
                        nc.tensor.matmul(g(pbn, j), ones_b[:, 0:BS],
                                         bhn[:, j, :], start=True, stop=False,
                                         tile_position=(0, 32 * j))
                    nrounds(KORD[:4])

                    # deferred: transpose prev step's hnew chunk 1 -> hTo
                    cast_c1 = None
                    if pend[0] is not None:
                        hnp, prev_hy1 = pend
                        ptr1 = ptrpool.tile([128, 4, 32], b16, tag="pt")
                        nc.tensor.transpose(ptr1[:, :, :], hnp[0:P4, C:2 * C],
                                            identTb[0:P4, :])
                        hTo_n = htopool.tile([128, 4, BS], b16, tag="hTo")
                        cast_c1 = after(
                            nc.vector.tensor_copy(hTo_n[:, :, :],
                                                  ptr1[:, :, 0:BS]),
                            prev_hy1, "cast_c1 after hy_c1")
                        hTo = hTo_n
                        pend = [None, None]

                    inject(1)
                    rounds(1, KORD[:4])

                    # ---- odds block: needs hTo (cast_c1 of t-1) ----
                    rounds(0, KORD[4:])          # pb0a stop
                    nrounds(KORD[4:])            # pbn stop
                    rounds(1, KORD[4:])          # pb0b stop

                    # ---- gate chains; engine-queue order pinned:
                    # ACT: sig0 -> sig1 -> tanh0 -> tanh1
                    # DVE: t1_0 -> t2_0 -> t1_1 -> t2_1 -> v2_0 -> hy_0
                    #      -> cast_c0 -> v2_1 -> hy_1 (cast_c1 next iter)
                    ri0 = gwork.tile([128, 2 * C], b16, tag="ri0")
                    sig0 = nc.scalar.activation(ri0[0:P4, :], pb0a[0:P4, :],
                                                AF.Sigmoid)
                    ri1 = gwork.tile([128, 2 * C], b16, tag="ri1")
                    sig1 = after(nc.scalar.activation(ri1[0:P4, :],
                                                      pb0b[0:P4, :], AF.Sigmoid),
                                 sig0, "sig1 after sig0")
                    ric = [ri0, ri1]

                    dve_prev = cast_c1
                    t2s, ngs, v3s = [], [], []
                    for c in range(2):
                        t1 = gwork.tile([128, C], b16, tag=f"t1{c}")
                        dve_prev = after(
                            nc.vector.tensor_mul(t1[0:P4, :], ric[c][0:P4, 0:C],
                                                 pbn[0:P4, C * c:C * (c + 1)]),
                            dve_prev, "dve order")
                        t2 = gwork.tile([128, C], b16, tag=f"t2{c}")
                        dve_prev = after(
                            nc.vector.tensor_add(
                                t2[0:P4, :], t1[0:P4, :],
                                gxt[0:P4, 2 * Q + C * c:2 * Q + C * (c + 1)]),
                            dve_prev, "dve order")
                        t2s.append(t2)
                    act_prev = sig1
                    for c in range(2):
                        ng = gwork.tile([128, C], b16, tag=f"ng{c}")
                        act_prev = after(
                            nc.scalar.activation(ng[0:P4, :], t2s[c][0:P4, :],
                                                 AF.Tanh),
                            act_prev, "tanh order")
                        ngs.append(ng)
                        # v3 = i * h_prev  (GpSimd, off critical path)
                        v3 = gwork.tile([128, C], b16, tag=f"v3{c}")
                        nc.gpsimd.tensor_mul(v3[0:P4, :], ric[c][0:P4, C:2 * C],
                                             hprev[0:P4, C * c:C * (c + 1)])
                        v3s.append(v3)

                    def upd(c, prev):
                        # v2' = (i - 1) * n  (fused);  h' = v3 - v2'
                        v2 = gwork.tile([128, C], b16, tag=f"v2{c}")
                        prev = after(
                            nc.vector.scalar_tensor_tensor(
                                v2[0:P4, :], ric[c][0:P4, C:2 * C], 1.0,
                                ngs[c][0:P4, :], op0=ALU.subtract, op1=ALU.mult),
                            prev, "dve order")
                        return after(
                            nc.vector.tensor_sub(
                                hnew[0:P4, C * c:C * (c + 1)],
                                v3s[c][0:P4, :], v2[0:P4, :]),
                            prev, "dve order")

                    dve_prev = upd(0, dve_prev)

                    # transpose + cast chunk 0 immediately (gates t+1 evens)
                    junk(3)
                    ptr0 = ptrpool.tile([128, 4, 32], b16, tag="pt")
                    nc.tensor.transpose(ptr0[:, :, :], hnew[0:P4, 0:C],
                                        identTb[0:P4, :])
                    hTe_n = htepool.tile([128, 4, BS], b16, tag="hTe")
                    dve_prev = after(
                        nc.vector.tensor_copy(hTe_n[:, :, :], ptr0[:, :, 0:BS]),
                        dve_prev, "cast_c0 after hy_c0")
                    hTe = hTe_n

                    hy1 = upd(1, dve_prev)
                    junk(1)
                    pend = [hnew, hy1]
                    hprev = hnew

                # flush the last deferred transpose
                if pend[0] is not None:
                    hnp, prev_hy1 = pend
                    ptr1 = ptrpool.tile([128, 4, 32], b16, tag="pt")
                    nc.tensor.transpose(ptr1[:, :, :], hnp[0:P4, C:2 * C],
                                        identTb[0:P4, :])
                    hTo_n = htopool.tile([128, 4, BS], b16, tag="hTo")
                    nc.vector.tensor_copy(hTo_n[:, :, :], ptr1[:, :, 0:BS])
                    hTo = hTo_n

                # ---------- final FC ----------
                po = ptrpool.tile([BS, OUT], f32, tag="pt")
                for k in range(8):
                    nc.tensor.matmul(po[:], hT_blk(k), wfcT[:, k, :],
                                     start=(k == 0), stop=False)
                nc.tensor.matmul(po[:], ones_f[:], bfc[:],
                                 start=False, stop=True)
                ob = gwork.tile([BS, OUT], f32, tag="ri0")
                nc.vector.tensor_copy(ob[:], po[:])
                nc.sync.dma_start(out_d[:], ob[:])

    nc.compile()
    return nc


def _prep_inputs(x, Wx, bx, Wh, bh, Wfc, bfc):
    """Host-side layout prep -> list of per-core input dicts."""
    import ml_dtypes
    bf16 = ml_dtypes.bfloat16

    x = np.asarray(x, np.float32)
    Wx = np.asarray(Wx, np.float32)
    bx = np.asarray(bx, np.float32)
    Wh = np.asarray(Wh, np.float32)
    bh = np.asarray(bh, np.float32)
    Wfc = np.asarray(Wfc, np.float32)
    bfc = np.asarray(bfc, np.float32)

    # grouped gate-column order per quarter j: [r0 | i0 | r1 | i1 | n(256)]
    # (rc/ic are 128-chunks of the quarter)
    blocks = []
    for j in range(4):
        base = j * Q
        for c in range(2):
            blocks.append(0 * H + base + c * C + np.arange(C))  # r chunk
            blocks.append(1 * H + base + c * C + np.arange(C))  # i chunk
        blocks.append(2 * H + base + np.arange(Q))              # n
    perm = np.concatenate(blocks)                 # [3072] grouped row index

    WxT_g = np.ascontiguousarray(Wx[perm].T)      # [256, 3072-grouped]
    bias_ri = bx + np.concatenate([bh[:2 * H], np.zeros(H, np.float32)])
    biasg = bias_ri[perm].reshape(1, H3)

    WhT = Wh.T                                     # [1024 hid, 3072 gates]
    whri = np.empty((8, 128, 4, 2, 2 * C), np.float32)
    whn = np.empty((8, 128, 4, Q), np.float32)
    for k in range(8):
        hid = slice(k * 128, (k + 1) * 128)
        for j in range(4):
            for c in range(2):
                q = slice(j * Q + c * C, j * Q + c * C + C)
                whri[k, :, j, c, :C] = WhT[hid, 0 * H:1 * H][:, q]
                whri[k, :, j, c, C:] = WhT[hid, 1 * H:2 * H][:, q]
            whn[k, :, j, :] = WhT[hid, 2 * H:3 * H][:, j * Q:(j + 1) * Q]
    bhn = bh[2 * H:].reshape(4, Q)[None, :, :]

    wfcT = np.ascontiguousarray(Wfc.T).reshape(8, 128, OUT)
    bfc2 = bfc.reshape(1, OUT)

    # transpose identity: diag 1 at columns where (col % 32) < 8, rows 0:104
    identT = np.zeros((128, 128), np.float32)
    for p in range(P4):
        if p % 32 < BS:
            identT[p, p] = 1.0

    common = {
        "wxg": WxT_g.reshape(2, 128, H3).astype(bf16),
        "biasg": biasg.astype(bf16),
        "whri": whri.astype(bf16),
        "whn": whn.astype(bf16),
        "bhn": bhn.astype(bf16),
        "wfcT": wfcT.astype(bf16),
        "bfc": bfc2,
        "identg": np.tile(np.vstack([np.eye(8, dtype=np.float32),
                                     np.zeros((24, 8), np.float32)]),
                          (4, 1)).astype(bf16),
        "identT": identT,
    }

    in_maps = []
    for cc in range(NCORES):
        xs = x[cc * BS:(cc + 1) * BS]             # [BS, T, IN]
        xsT = xs.transpose(2, 1, 0)               # [IN, T, BS]
        in_maps.append({
            "xsT": np.ascontiguousarray(xsT.reshape(2, 128, T * BS)).astype(bf16),
            **common,
        })
    return in_maps


def kernel(x, Wx, bx, Wh, bh, Wfc, bfc):
    from concourse.bass_utils import run_bass_kernel_spmd

    if "nc" not in _cache:
        _cache["nc"] = _build_program()
    nc = _cache["nc"]

    in_maps = _prep_inputs(x, Wx, bx, Wh, bh, Wfc, bfc)
    res = run_bass_kernel_spmd(nc, in_maps, list(range(NCORES)))
    out = np.concatenate([res.results[c]["out"] for c in range(NCORES)], axis=0)
    return out.astype(np.float32)


if __name__ == "__main__":
    rng = np.random.default_rng(0)
    std = 1.0 / np.sqrt(H)
    inputs = {
        "x": rng.standard_normal((B, T, IN), dtype=np.float32),
        "Wx": rng.uniform(-std, std, (H3, IN)).astype(np.float32),
        "bx": rng.uniform(-std, std, (H3,)).astype(np.float32),
        "Wh": rng.uniform(-std, std, (H3, H)).astype(np.float32),
        "bh": rng.uniform(-std, std, (H3,)).astype(np.float32),
        "Wfc": rng.uniform(-std, std, (OUT, H)).astype(np.float32),
        "bfc": rng.uniform(-std, std, (OUT,)).astype(np.float32),
    }
    out = kernel(**inputs)
    print("out", out.shape, out.dtype)
    print(out[:2])


# revision 24
# speedup vs baseline: 1.0815x; 1.0815x over previous
"""GRU kernel for Trainium2 (8 NeuronCores, SPMD).

Problem: B=64, T=512, IN=256, H=1024, OUT=10
  gate_x_all = x @ Wx.T + bx            [B,T,3H]
  scan over T:  gate_h = h @ Wh.T + bh
                r = sig(i_r + h_r); i = sig(i_i + h_i)
                n = tanh(i_n + r * h_n)
                h = n + i * (h - n)
  out = h_last @ Wfc.T + bfc            [B,OUT]

Strategy (v10): data-parallel over batch (8 rows/core), weights replicated.
 - bf16 operands everywhere (incl. the h state), fp32 PSUM accumulation.
 - Phase 1: gate_x precomputed for all T in one large weight-streaming GEMM.
 - Phase 2: the T-step scan, software-pipelined in 2 feature-chunks:
   * gate columns per hidden-quarter j ordered [r0|i0|r1|i1|n] (128-chunks)
     so ONE sigmoid covers a chunk's r and i halves.
   * per-step PSUM: pbn (n-gate) + pb0c0/pb0c1 (r,i per chunk) with separate
     accumulation groups -> chunk-0's sigmoid/tanh chain runs while chunk-1
     matmuls still stream.
   * h update: i=sig(.); v2'=(i-1)*tanh-gate (fused); v3=i*h (GpSimd);
     h' = v3 - v2'.
   * h' -> hT via 2 full-width PE transposes (one per 128-chunk), cast into
     separate even/odd hT tiles; next step's matmul k-rounds run evens first
     so streaming restarts before the second chunk's chain finishes.
"""

import os
import sys

import numpy as np

for p in ("/root/.axon_site", "/root/.axon_site/_ro/trn_rl_repo",
          "/root/.axon_site/_ro/pypackages", "/opt/trn_rl_repo"):
    if p not in sys.path:
        sys.path.append(p)

B, T, IN, H, OUT = 64, 512, 256, 1024, 10
H3 = 3 * H
NCORES = 8
BS = B // NCORES      # batch per core (8)
Q = H // 4            # hidden quarter (256)
GF = 3 * Q            # per-group gate columns (768): [r0 i0 r1 i1 | n]
C = 128               # feature chunk within a quarter
P4 = 3 * 32 + BS      # 104: partitions covering all 4 batch groups

_cache = {}


def _build_program():
    import concourse.bass as bass
    import concourse.bacc as bacc
    import concourse.mybir as mybir
    from concourse.tile import TileContext
    from concourse.tile_rust import add_dep_helper

    def after(inst, prev, why="order"):
        # pin engine-queue order (sync=False: no semaphore, FIFO suffices)
        if prev is not None:
            add_dep_helper(inst.ins, prev.ins, sync=False, reason=why)
        return inst

    f32 = mybir.dt.float32
    b16 = mybir.dt.bfloat16
    AF = mybir.ActivationFunctionType
    ALU = mybir.AluOpType

    nc = bacc.Bacc(None, target_bir_lowering=False)

    # ---- per-core inputs ----
    xsT_d = nc.dram_tensor("xsT", [2, 128, T * BS], b16, kind="ExternalInput")
    wxg_d = nc.dram_tensor("wxg", [2, 128, H3], b16, kind="ExternalInput")
    biasg_d = nc.dram_tensor("biasg", [1, H3], b16, kind="ExternalInput")
    # Wh^T: r,i interleaved by 128-chunk [8k,128,4j,2c,256(r|i)], n separate
    whri_d = nc.dram_tensor("whri", [8, 128, 4, 2, 2 * C], b16, kind="ExternalInput")
    whn_d = nc.dram_tensor("whn", [8, 128, 4, Q], b16, kind="ExternalInput")
    bhn_d = nc.dram_tensor("bhn", [1, 4, Q], b16, kind="ExternalInput")
    wfcT_d = nc.dram_tensor("wfcT", [8, 128, OUT], b16, kind="ExternalInput")
    bfc_d = nc.dram_tensor("bfc", [1, OUT], f32, kind="ExternalInput")
    ident_d = nc.dram_tensor("identg", [128, 8], b16, kind="ExternalInput")
    identT_d = nc.dram_tensor("identT", [128, 128], f32, kind="ExternalInput")
    out_d = nc.dram_tensor("out", [BS, OUT], f32, kind="ExternalOutput")

    # gate_x for all T: [m=T/16, p=(16t x 8b), 4*GF] in grouped column order
    gx_d = nc.dram_tensor("gx_all", [T // 16, 128, H3], b16)

    t_steps = int(os.environ.get("KERNEL_T", str(T)))
    KORD = (0, 2, 4, 6, 1, 3, 5, 7)   # evens first: hTe ready before hTo

    with TileContext(nc) as tc:
        # ---------- constants ----------
        with tc.tile_pool(name="const", bufs=1) as cpool:
            whri = cpool.tile([128, 8, 4, 2, 2 * C], b16)
            for k in range(8):
                for j in range(4):
                    nc.sync.dma_start(whri[:, k, j, :, :], whri_d[k, :, j, :, :])
            whn = cpool.tile([128, 8, 4, Q], b16)
            for k in range(8):
                for j in range(4):
                    nc.sync.dma_start(whn[:, k, j, :], whn_d[k, :, j, :])
            bhn = cpool.tile([1, 4, Q], b16)
            nc.sync.dma_start(bhn[:], bhn_d[:])
            wfcT = cpool.tile([128, 8, OUT], b16)
            for k in range(8):
                nc.sync.dma_start(wfcT[:, k, :], wfcT_d[k])
            bfc = cpool.tile([1, OUT], f32)
            nc.sync.dma_start(bfc[:], bfc_d[:])
            ones_b = cpool.tile([1, 128], b16)
            nc.gpsimd.memset(ones_b[:], 1.0)
            ones_f = cpool.tile([1, BS], f32)
            nc.gpsimd.memset(ones_f[:], 1.0)
            identg = cpool.tile([128, 8], b16)
            nc.sync.dma_start(identg[:], ident_d[:])
            identT = cpool.tile([128, 128], f32)
            nc.sync.dma_start(identT[:], identT_d[:])
            identTb = cpool.tile([128, 128], b16)
            nc.vector.tensor_copy(identTb[:], identT[:])

            # ---------- phase 1: gate_x precompute ----------
            with (
                tc.tile_pool(name="px", bufs=2) as pxpool,
                tc.tile_pool(name="pxo", bufs=4) as pxopool,
                tc.tile_pool(name="ppre", bufs=3, space="PSUM") as ppre,
            ):
                xsT = pxpool.tile([128, 2, T * BS], b16)
                for k in range(2):
                    for c in range(8):
                        cs = slice(c * 512, (c + 1) * 512)
                        nc.sync.dma_start(xsT[:, k, cs], xsT_d[k, :, cs])
                wxg = pxpool.tile([128, 2, H3], b16)
                for k in range(2):
                    for c in range(6):
                        cs = slice(c * 512, (c + 1) * 512)
                        nc.sync.dma_start(wxg[:, k, cs], wxg_d[k, :, cs])
                biasg = pxpool.tile([1, H3], b16)
                nc.sync.dma_start(biasg[:], biasg_d[:])

                for m in range(32):          # M-tiles: 128 rows = 16 t x 8 b
                    ms = slice(m * 128, (m + 1) * 128)
                    for j in range(4):       # N-chunks of GF=768 (one group)
                        ncs = slice(j * GF, (j + 1) * GF)
                        pp = ppre.tile([128, GF], f32, tag="pp")
                        for k in range(2):
                            for (o, w) in ((0, 512), (512, 256)):
                                nc.tensor.matmul(
                                    pp[:, o:o + w], xsT[:, k, ms],
                                    wxg[:, k, j * GF + o:j * GF + o + w],
                                    start=(k == 0), stop=False)
                        for (o, w) in ((0, 512), (512, 256)):
                            nc.tensor.matmul(
                                pp[:, o:o + w], ones_b[:],
                                biasg[:, j * GF + o:j * GF + o + w],
                                start=False, stop=True)
                        ob = pxopool.tile([128, GF], b16, tag="ob")
                        if j % 2 == 0:
                            nc.vector.tensor_copy(ob[:], pp[:])
                            nc.sync.dma_start(gx_d[m, :, ncs], ob[:])
                        else:
                            nc.scalar.activation(ob[:], pp[:], AF.Copy)
                            nc.scalar.dma_start(gx_d[m, :, ncs], ob[:])

            # ---------- phase 2: the scan ----------
            with (
                tc.tile_pool(name="state", bufs=3) as spool,
                tc.tile_pool(name="hte", bufs=2) as htepool,
                tc.tile_pool(name="hto", bufs=2) as htopool,
                tc.tile_pool(name="gxt", bufs=6) as gxpool,
                tc.tile_pool(name="gw", bufs=3) as gwork,
                tc.tile_pool(name="pbn", bufs=1, space="PSUM") as pbnpool,
                tc.tile_pool(name="pba", bufs=2, space="PSUM") as pbapool,
                tc.tile_pool(name="pbb", bufs=2, space="PSUM") as pbbpool,
                tc.tile_pool(name="ptr", bufs=2, space="PSUM") as ptrpool,
                tc.tile_pool(name="pwm", bufs=1, space="PSUM") as pwmpool,
            ):
                # HAM keep-warm target: junk matmuls during chain waits keep
                # the PE clock at 8/8 (sub-us idle gaps otherwise oscillate it)
                pwarm = pwmpool.tile([128, 512], f32, tag="pw")
                hTe = htepool.tile([128, 4, BS], b16, tag="hTe")
                nc.gpsimd.memset(hTe[:], 0.0)
                hTo = htopool.tile([128, 4, BS], b16, tag="hTo")
                nc.gpsimd.memset(hTo[:], 0.0)
                hprev = spool.tile([128, Q], b16, tag="h")
                nc.gpsimd.memset(hprev[:], 0.0)

                def g(ap, j):
                    return ap[32 * j:32 * j + BS]

                def hT_blk(k):
                    src = hTe if k % 2 == 0 else hTo
                    return src[:, k // 2, :]

                def junk(n):
                    # deps-free PE filler (constant lhsT, constant rhs)
                    for w in range(n):
                        nc.tensor.matmul(pwarm[0:BS, :], identg[:, 0:BS],
                                         whri[:, w % 8, 0, :, :],
                                         start=True, stop=True)

                # previous step's hnew (chunk-1 transpose deferred into the
                # next step's evens block) + its last DVE chain op
                pend = [None, None]   # (hnew_tile, hy_c1_inst)

                for t in range(t_steps):
                    m, tt = t // 16, t % 16
                    gxt = gxpool.tile([128, GF], b16, tag="gxt")
                    for j in range(4):
                        nc.sync.dma_start(
                            gxt[32 * j:32 * j + BS, :],
                            gx_d[m, tt * BS:(tt + 1) * BS,
                                 j * GF:(j + 1) * GF])

                    pbn = pbnpool.tile([128, Q], f32, tag="pbn")
                    pb0a = pbapool.tile([128, 2 * C], f32, tag="pa")
                    pb0b = pbbpool.tile([128, 2 * C], f32, tag="pb")
                    pb0 = [pb0a, pb0b]
                    hnew = spool.tile([128, Q], b16, tag="h")

                    def inject(c):
                        for j in range(4):
                            nc.tensor.matmul(
                                g(pb0[c], j), g(identg, j),
                                gxt[32 * j:32 * j + BS, 2 * C * c:2 * C * (c + 1)],
                                start=True, stop=False,
                                tile_position=(32 * j, 32 * j))

                    def rounds(c, ks):
                        for k in ks:
                            for j in range(4):
                                nc.tensor.matmul(g(pb0[c], j), hT_blk(k),
                                                 whri[:, k, j, c, :],
                                                 start=False, stop=(k == 7),
                                                 tile_position=(0, 32 * j))

                    def nrounds(ks):
                        for k in ks:
                            for j in range(4):
                                nc.tensor.matmul(g(pbn, j), hT_blk(k),
                                                 whn[:, k, j, :],
                                                 start=False, stop=(k == 7),
                                                 tile_position=(0, 32 * j))

                    # ---- evens block: needs only hTe (cast_c0 of t-1) ----
                    junk(2)
                    inject(0)
                    rounds(0, KORD[:4])

                    # deferred: transpose prev step's hnew chunk 1 -> hTo
                    cast_c1 = None
                    if pend[0] is not None:
                        hnp, prev_hy1 = pend
                        ptr1 = ptrpool.tile([128, 4, 32], b16, tag="pt")
                        nc.tensor.transpose(ptr1[:, :, :], hnp[0:P4, C:2 * C],
                                            identTb[0:P4, :])
                        hTo_n = htopool.tile([128, 4, BS], b16, tag="hTo")
                        cast_c1 = after(
                            nc.vector.tensor_copy(hTo_n[:, :, :],
                                                  ptr1[:, :, 0:BS]),
                            prev_hy1, "cast_c1 after hy_c1")
                        hTo = hTo_n
                        pend = [None, None]

                    for j in range(4):
                        nc.tensor.matmul(g(pbn, j), ones_b[:, 0:BS],
                                         bhn[:, j, :], start=True, stop=False,
                                         tile_position=(0, 32 * j))
                    nrounds(KORD[:4])
                    inject(1)
                    rounds(1, KORD[:4])

                    # ---- odds block: needs hTo (cast_c1 of t-1) ----
                    rounds(0, KORD[4:])          # pb0a stop
                    nrounds(KORD[4:])            # pbn stop
                    rounds(1, KORD[4:])          # pb0b stop

                    # ---- gate chains; engine-queue order pinned:
                    # ACT: sig0 -> sig1 -> tanh0 -> tanh1
                    # DVE: t1_0 -> t2_0 -> t1_1 -> t2_1 -> v2_0 -> hy_0
                    #      -> cast_c0 -> v2_1 -> hy_1 (cast_c1 next iter)
                    ri0 = gwork.tile([128, 2 * C], b16, tag="ri0")
                    sig0 = nc.scalar.activation(ri0[0:P4, :], pb0a[0:P4, :],
                                                AF.Sigmoid)
                    ri1 = gwork.tile([128, 2 * C], b16, tag="ri1")
                    sig1 = after(nc.scalar.activation(ri1[0:P4, :],
                                                      pb0b[0:P4, :], AF.Sigmoid),
                                 sig0, "sig1 after sig0")
                    ric = [ri0, ri1]

                    dve_prev = cast_c1
                    t2s, ngs, v3s = [], [], []
                    for c in range(2):
                        t1 = gwork.tile([128, C], b16, tag=f"t1{c}")
                        dve_prev = after(
                            nc.vector.tensor_mul(t1[0:P4, :], ric[c][0:P4, 0:C],
                                                 pbn[0:P4, C * c:C * (c + 1)]),
                            dve_prev, "dve order")
                        t2 = gwork.tile([128, C], b16, tag=f"t2{c}")
                        dve_prev = after(
                            nc.vector.tensor_add(
                                t2[0:P4, :], t1[0:P4, :],
                                gxt[0:P4, 2 * Q + C * c:2 * Q + C * (c + 1)]),
                            dve_prev, "dve order")
                        t2s.append(t2)
                    act_prev = sig1
                    for c in range(2):
                        ng = gwork.tile([128, C], b16, tag=f"ng{c}")
                        act_prev = after(
                            nc.scalar.activation(ng[0:P4, :], t2s[c][0:P4, :],
                                                 AF.Tanh),
                            act_prev, "tanh order")
                        ngs.append(ng)
                        # v3 = i * h_prev  (GpSimd, off critical path)
                        v3 = gwork.tile([128, C], b16, tag=f"v3{c}")
                        nc.gpsimd.tensor_mul(v3[0:P4, :], ric[c][0:P4, C:2 * C],
                                             hprev[0:P4, C * c:C * (c + 1)])
                        v3s.append(v3)

                    def upd(c, prev):
                        # v2' = (i - 1) * n  (fused);  h' = v3 - v2'
                        v2 = gwork.tile([128, C], b16, tag=f"v2{c}")
                        prev = after(
                            nc.vector.scalar_tensor_tensor(
                                v2[0:P4, :], ric[c][0:P4, C:2 * C], 1.0,
                                ngs[c][0:P4, :], op0=ALU.subtract, op1=ALU.mult),
                            prev, "dve order")
                        return after(
                            nc.vector.tensor_sub(
                                hnew[0:P4, C * c:C * (c + 1)],
                                v3s[c][0:P4, :], v2[0:P4, :]),
                            prev, "dve order")

                    dve_prev = upd(0, dve_prev)

                    # transpose chunk 0 immediately; its cast runs on the
                    # Scalar engine (idle after tanh1) so DVE can finish the
                    # chunk-1 chain without queuing behind it
                    junk(3)
                    ptr0 = ptrpool.tile([128, 4, 32], b16, tag="pt")
                    nc.tensor.transpose(ptr0[:, :, :], hnew[0:P4, 0:C],
                                        identTb[0:P4, :])
                    hTe_n = htepool.tile([128, 4, BS], b16, tag="hTe")
                    act_prev = after(
                        nc.scalar.activation(hTe_n[:, :, :], ptr0[:, :, 0:BS],
                                             AF.Copy),
                        act_prev, "cast_c0 on ACT after tanh1")
                    hTe = hTe_n

                    hy1 = upd(1, dve_prev)
                    junk(1)
                    pend = [hnew, hy1]
                    hprev = hnew

                # flush the last deferred transpose
                if pend[0] is not None:
                    hnp, prev_hy1 = pend
                    ptr1 = ptrpool.tile([128, 4, 32], b16, tag="pt")
                    nc.tensor.transpose(ptr1[:, :, :], hnp[0:P4, C:2 * C],
                                        identTb[0:P4, :])
                    hTo_n = htopool.tile([128, 4, BS], b16, tag="hTo")
                    nc.vector.tensor_copy(hTo_n[:, :, :], ptr1[:, :, 0:BS])
                    hTo = hTo_n

                # ---------- final FC ----------
                po = ptrpool.tile([BS, OUT], f32, tag="pt")
                for k in range(8):
                    nc.tensor.matmul(po[:], hT_blk(k), wfcT[:, k, :],
                                     start=(k == 0), stop=False)
                nc.tensor.matmul(po[:], ones_f[:], bfc[:],
                                 start=False, stop=True)
                ob = gwork.tile([BS, OUT], f32, tag="ri0")
                nc.vector.tensor_copy(ob[:], po[:])
                nc.sync.dma_start(out_d[:], ob[:])

    nc.compile()
    return nc


def _prep_inputs(x, Wx, bx, Wh, bh, Wfc, bfc):
    """Host-side layout prep -> list of per-core input dicts."""
    import ml_dtypes
    bf16 = ml_dtypes.bfloat16

    x = np.asarray(x, np.float32)
    Wx = np.asarray(Wx, np.float32)
    bx = np.asarray(bx, np.float32)
    Wh = np.asarray(Wh, np.float32)
    bh = np.asarray(bh, np.float32)
    Wfc = np.asarray(Wfc, np.float32)
    bfc = np.asarray(bfc, np.float32)

    # grouped gate-column order per quarter j: [r0 | i0 | r1 | i1 | n(256)]
    # (rc/ic are 128-chunks of the quarter)
    blocks = []
    for j in range(4):
        base = j * Q
        for c in range(2):
            blocks.append(0 * H + base + c * C + np.arange(C))  # r chunk
            blocks.append(1 * H + base + c * C + np.arange(C))  # i chunk
        blocks.append(2 * H + base + np.arange(Q))              # n
    perm = np.concatenate(blocks)                 # [3072] grouped row index

    WxT_g = np.ascontiguousarray(Wx[perm].T)      # [256, 3072-grouped]
    bias_ri = bx + np.concatenate([bh[:2 * H], np.zeros(H, np.float32)])
    biasg = bias_ri[perm].reshape(1, H3)

    WhT = Wh.T                                     # [1024 hid, 3072 gates]
    whri = np.empty((8, 128, 4, 2, 2 * C), np.float32)
    whn = np.empty((8, 128, 4, Q), np.float32)
    for k in range(8):
        hid = slice(k * 128, (k + 1) * 128)
        for j in range(4):
            for c in range(2):
                q = slice(j * Q + c * C, j * Q + c * C + C)
                whri[k, :, j, c, :C] = WhT[hid, 0 * H:1 * H][:, q]
                whri[k, :, j, c, C:] = WhT[hid, 1 * H:2 * H][:, q]
            whn[k, :, j, :] = WhT[hid, 2 * H:3 * H][:, j * Q:(j + 1) * Q]
    bhn = bh[2 * H:].reshape(4, Q)[None, :, :]

    wfcT = np.ascontiguousarray(Wfc.T).reshape(8, 128, OUT)
    bfc2 = bfc.reshape(1, OUT)

    # transpose identity: diag 1 at columns where (col % 32) < 8, rows 0:104
    identT = np.zeros((128, 128), np.float32)
    for p in range(P4):
        if p % 32 < BS:
            identT[p, p] = 1.0

    common = {
        "wxg": WxT_g.reshape(2, 128, H3).astype(bf16),
        "biasg": biasg.astype(bf16),
        "whri": whri.astype(bf16),
        "whn": whn.astype(bf16),
        "bhn": bhn.astype(bf16),
        "wfcT": wfcT.astype(bf16),
        "bfc": bfc2,
        "identg": np.tile(np.vstack([np.eye(8, dtype=np.float32),
                                     np.zeros((24, 8), np.float32)]),
                          (4, 1)).astype(bf16),
        "identT": identT,
    }

    in_maps = []
    for cc in range(NCORES):
        xs = x[cc * BS:(cc + 1) * BS]             # [BS, T, IN]
        xsT = xs.transpose(2, 1, 0)               # [IN, T, BS]
        in_maps.append({
            "xsT": np.ascontiguousarray(xsT.reshape(2, 128, T * BS)).astype(bf16),
            **common,
        })
    return in_maps


def kernel(x, Wx, bx, Wh, bh, Wfc, bfc):
    from concourse.bass_utils import run_bass_kernel_spmd

    if "nc" not in _cache:
        _cache["nc"] = _build_program()
    nc = _cache["nc"]

    in_maps = _prep_inputs(x, Wx, bx, Wh, bh, Wfc, bfc)
    res = run_bass_kernel_spmd(nc, in_maps, list(range(NCORES)))
    out = np.concatenate([res.results[c]["out"] for c in range(NCORES)], axis=0)
    return out.astype(np.float32)


if __name__ == "__main__":
    rng = np.random.default_rng(0)
    std = 1.0 / np.sqrt(H)
    inputs = {
        "x": rng.standard_normal((B, T, IN), dtype=np.float32),
        "Wx": rng.uniform(-std, std, (H3, IN)).astype(np.float32),
        "bx": rng.uniform(-std, std, (H3,)).astype(np.float32),
        "Wh": rng.uniform(-std, std, (H3, H)).astype(np.float32),
        "bh": rng.uniform(-std, std, (H3,)).astype(np.float32),
        "Wfc": rng.uniform(-std, std, (OUT, H)).astype(np.float32),
        "bfc": rng.uniform(-std, std, (OUT,)).astype(np.float32),
    }
    out = kernel(**inputs)
    print("out", out.shape, out.dtype)
    print(out[:2])


# revision 25
# speedup vs baseline: 1.1154x; 1.0313x over previous
"""GRU kernel for Trainium2 (8 NeuronCores, SPMD).

Problem: B=64, T=512, IN=256, H=1024, OUT=10
  gate_x_all = x @ Wx.T + bx            [B,T,3H]
  scan over T:  gate_h = h @ Wh.T + bh
                r = sig(i_r + h_r); i = sig(i_i + h_i)
                n = tanh(i_n + r * h_n)
                h = n + i * (h - n)
  out = h_last @ Wfc.T + bfc            [B,OUT]

Strategy (v10): data-parallel over batch (8 rows/core), weights replicated.
 - bf16 operands everywhere (incl. the h state), fp32 PSUM accumulation.
 - Phase 1: gate_x precomputed for all T in one large weight-streaming GEMM.
 - Phase 2: the T-step scan, software-pipelined in 2 feature-chunks:
   * gate columns per hidden-quarter j ordered [r0|i0|r1|i1|n] (128-chunks)
     so ONE sigmoid covers a chunk's r and i halves.
   * per-step PSUM: pbn (n-gate) + pb0c0/pb0c1 (r,i per chunk) with separate
     accumulation groups -> chunk-0's sigmoid/tanh chain runs while chunk-1
     matmuls still stream.
   * h update: i=sig(.); v2'=(i-1)*tanh-gate (fused); v3=i*h (GpSimd);
     h' = v3 - v2'.
   * h' -> hT via 2 full-width PE transposes (one per 128-chunk), cast into
     separate even/odd hT tiles; next step's matmul k-rounds run evens first
     so streaming restarts before the second chunk's chain finishes.
"""

import os
import sys

import numpy as np

for p in ("/root/.axon_site", "/root/.axon_site/_ro/trn_rl_repo",
          "/root/.axon_site/_ro/pypackages", "/opt/trn_rl_repo"):
    if p not in sys.path:
        sys.path.append(p)

B, T, IN, H, OUT = 64, 512, 256, 1024, 10
H3 = 3 * H
NCORES = 8
BS = B // NCORES      # batch per core (8)
Q = H // 4            # hidden quarter (256)
GF = 3 * Q            # per-group gate columns (768): [r0 i0 r1 i1 | n]
C = 128               # feature chunk within a quarter
P4 = 3 * 32 + BS      # 104: partitions covering all 4 batch groups

_cache = {}


def _build_program():
    import concourse.bass as bass
    import concourse.bacc as bacc
    import concourse.mybir as mybir
    from concourse.tile import TileContext
    from concourse.tile_rust import add_dep_helper

    def after(inst, prev, why="order"):
        # pin engine-queue order (sync=False: no semaphore, FIFO suffices)
        if prev is not None:
            add_dep_helper(inst.ins, prev.ins, sync=False, reason=why)
        return inst

    f32 = mybir.dt.float32
    b16 = mybir.dt.bfloat16
    AF = mybir.ActivationFunctionType
    ALU = mybir.AluOpType

    nc = bacc.Bacc(None, target_bir_lowering=False)

    # ---- per-core inputs ----
    xsT_d = nc.dram_tensor("xsT", [2, 128, T * BS], b16, kind="ExternalInput")
    wxg_d = nc.dram_tensor("wxg", [2, 128, H3], b16, kind="ExternalInput")
    biasg_d = nc.dram_tensor("biasg", [1, H3], b16, kind="ExternalInput")
    # Wh^T: r,i interleaved by 128-chunk [8k,128,4j,2c,256(r|i)], n separate
    whri_d = nc.dram_tensor("whri", [8, 128, 4, 2, 2 * C], b16, kind="ExternalInput")
    whn_d = nc.dram_tensor("whn", [8, 128, 4, Q], b16, kind="ExternalInput")
    bhn_d = nc.dram_tensor("bhn", [1, 4, Q], b16, kind="ExternalInput")
    wfcT_d = nc.dram_tensor("wfcT", [8, 128, OUT], b16, kind="ExternalInput")
    bfc_d = nc.dram_tensor("bfc", [1, OUT], f32, kind="ExternalInput")
    ident_d = nc.dram_tensor("identg", [128, 8], b16, kind="ExternalInput")
    identT_d = nc.dram_tensor("identT", [128, 128], f32, kind="ExternalInput")
    out_d = nc.dram_tensor("out", [BS, OUT], f32, kind="ExternalOutput")

    # gate_x for all T: [m=T/16, p=(16t x 8b), 4*GF] in grouped column order
    gx_d = nc.dram_tensor("gx_all", [T // 16, 128, H3], b16)

    t_steps = int(os.environ.get("KERNEL_T", str(T)))
    KORD = (0, 2, 4, 6, 1, 3, 5, 7)   # evens first: hTe ready before hTo

    with TileContext(nc) as tc:
        # ---------- constants ----------
        with tc.tile_pool(name="const", bufs=1) as cpool:
            whri = cpool.tile([128, 8, 4, 2, 2 * C], b16)
            for k in range(8):
                for j in range(4):
                    nc.sync.dma_start(whri[:, k, j, :, :], whri_d[k, :, j, :, :])
            whn = cpool.tile([128, 8, 4, Q], b16)
            for k in range(8):
                for j in range(4):
                    nc.sync.dma_start(whn[:, k, j, :], whn_d[k, :, j, :])
            bhn = cpool.tile([1, 4, Q], b16)
            nc.sync.dma_start(bhn[:], bhn_d[:])
            wfcT = cpool.tile([128, 8, OUT], b16)
            for k in range(8):
                nc.sync.dma_start(wfcT[:, k, :], wfcT_d[k])
            bfc = cpool.tile([1, OUT], f32)
            nc.sync.dma_start(bfc[:], bfc_d[:])
            ones_b = cpool.tile([1, 128], b16)
            nc.gpsimd.memset(ones_b[:], 1.0)
            ones_f = cpool.tile([1, BS], f32)
            nc.gpsimd.memset(ones_f[:], 1.0)
            identg = cpool.tile([128, 8], b16)
            nc.sync.dma_start(identg[:], ident_d[:])
            identT = cpool.tile([128, 128], f32)
            nc.sync.dma_start(identT[:], identT_d[:])
            identTb = cpool.tile([128, 128], b16)
            nc.vector.tensor_copy(identTb[:], identT[:])

            # ---------- phase 1: gate_x precompute ----------
            with (
                tc.tile_pool(name="px", bufs=2) as pxpool,
                tc.tile_pool(name="pxo", bufs=4) as pxopool,
                tc.tile_pool(name="ppre", bufs=3, space="PSUM") as ppre,
            ):
                xsT = pxpool.tile([128, 2, T * BS], b16)
                for k in range(2):
                    for c in range(8):
                        cs = slice(c * 512, (c + 1) * 512)
                        nc.sync.dma_start(xsT[:, k, cs], xsT_d[k, :, cs])
                wxg = pxpool.tile([128, 2, H3], b16)
                for k in range(2):
                    for c in range(6):
                        cs = slice(c * 512, (c + 1) * 512)
                        nc.sync.dma_start(wxg[:, k, cs], wxg_d[k, :, cs])
                biasg = pxpool.tile([1, H3], b16)
                nc.sync.dma_start(biasg[:], biasg_d[:])

                for m in range(32):          # M-tiles: 128 rows = 16 t x 8 b
                    ms = slice(m * 128, (m + 1) * 128)
                    for j in range(4):       # N-chunks of GF=768 (one group)
                        ncs = slice(j * GF, (j + 1) * GF)
                        pp = ppre.tile([128, GF], f32, tag="pp")
                        for k in range(2):
                            for (o, w) in ((0, 512), (512, 256)):
                                nc.tensor.matmul(
                                    pp[:, o:o + w], xsT[:, k, ms],
                                    wxg[:, k, j * GF + o:j * GF + o + w],
                                    start=(k == 0), stop=False)
                        for (o, w) in ((0, 512), (512, 256)):
                            nc.tensor.matmul(
                                pp[:, o:o + w], ones_b[:],
                                biasg[:, j * GF + o:j * GF + o + w],
                                start=False, stop=True)
                        ob = pxopool.tile([128, GF], b16, tag="ob")
                        if j % 2 == 0:
                            nc.vector.tensor_copy(ob[:], pp[:])
                            nc.sync.dma_start(gx_d[m, :, ncs], ob[:])
                        else:
                            nc.scalar.activation(ob[:], pp[:], AF.Copy)
                            nc.scalar.dma_start(gx_d[m, :, ncs], ob[:])

            # ---------- phase 2: the scan ----------
            with (
                tc.tile_pool(name="state", bufs=3) as spool,
                tc.tile_pool(name="hte", bufs=2) as htepool,
                tc.tile_pool(name="hto", bufs=2) as htopool,
                tc.tile_pool(name="gxt", bufs=6) as gxpool,
                tc.tile_pool(name="gw", bufs=3) as gwork,
                tc.tile_pool(name="pbn", bufs=1, space="PSUM") as pbnpool,
                tc.tile_pool(name="pba", bufs=2, space="PSUM") as pbapool,
                tc.tile_pool(name="pbb", bufs=2, space="PSUM") as pbbpool,
                tc.tile_pool(name="ptr", bufs=2, space="PSUM") as ptrpool,
                tc.tile_pool(name="pwm", bufs=1, space="PSUM") as pwmpool,
            ):
                # HAM keep-warm target: junk matmuls during chain waits keep
                # the PE clock at 8/8 (sub-us idle gaps otherwise oscillate it)
                pwarm = pwmpool.tile([128, 512], f32, tag="pw")
                hTe = htepool.tile([128, 4, BS], b16, tag="hTe")
                nc.gpsimd.memset(hTe[:], 0.0)
                hTo = htopool.tile([128, 4, BS], b16, tag="hTo")
                nc.gpsimd.memset(hTo[:], 0.0)
                hprev = spool.tile([128, Q], b16, tag="h")
                nc.gpsimd.memset(hprev[:], 0.0)

                def g(ap, j):
                    return ap[32 * j:32 * j + BS]

                def hT_blk(k):
                    src = hTe if k % 2 == 0 else hTo
                    return src[:, k // 2, :]

                def junk(n):
                    # deps-free PE filler (constant lhsT, constant rhs)
                    for w in range(n):
                        nc.tensor.matmul(pwarm[0:BS, :], identg[:, 0:BS],
                                         whri[:, w % 8, 0, :, :],
                                         start=True, stop=True)

                # previous step's hnew (chunk-1 transpose deferred into the
                # next step's evens block) + its last DVE chain op
                pend = [None, None]   # (hnew_tile, hy_c1_inst)

                for t in range(t_steps):
                    m, tt = t // 16, t % 16
                    gxt = gxpool.tile([128, GF], b16, tag="gxt")
                    for j in range(4):
                        nc.sync.dma_start(
                            gxt[32 * j:32 * j + BS, :],
                            gx_d[m, tt * BS:(tt + 1) * BS,
                                 j * GF:(j + 1) * GF])

                    pbn = pbnpool.tile([128, Q], f32, tag="pbn")
                    pb0a = pbapool.tile([128, 2 * C], f32, tag="pa")
                    pb0b = pbbpool.tile([128, 2 * C], f32, tag="pb")
                    pb0 = [pb0a, pb0b]
                    hnew = spool.tile([128, Q], b16, tag="h")

                    def inject(c):
                        for j in range(4):
                            nc.tensor.matmul(
                                g(pb0[c], j), g(identg, j),
                                gxt[32 * j:32 * j + BS, 2 * C * c:2 * C * (c + 1)],
                                start=True, stop=False,
                                tile_position=(32 * j, 32 * j))

                    def rounds(c, ks):
                        for k in ks:
                            for j in range(4):
                                nc.tensor.matmul(g(pb0[c], j), hT_blk(k),
                                                 whri[:, k, j, c, :],
                                                 start=False, stop=(k == 7),
                                                 tile_position=(0, 32 * j))

                    def nrounds(ks):
                        for k in ks:
                            for j in range(4):
                                nc.tensor.matmul(g(pbn, j), hT_blk(k),
                                                 whn[:, k, j, :],
                                                 start=False, stop=(k == 7),
                                                 tile_position=(0, 32 * j))

                    # ---- evens block: needs only hTe (cast_c0 of t-1) ----
                    junk(2)
                    inject(0)
                    rounds(0, KORD[:4])

                    # deferred: transpose prev step's hnew chunk 1 -> hTo
                    cast_c1 = None
                    if pend[0] is not None:
                        hnp, prev_hy1 = pend
                        ptr1 = ptrpool.tile([128, 4, 32], b16, tag="pt")
                        nc.tensor.transpose(ptr1[:, :, :], hnp[0:P4, C:2 * C],
                                            identTb[0:P4, :])
                        hTo_n = htopool.tile([128, 4, BS], b16, tag="hTo")
                        cast_c1 = after(
                            nc.vector.tensor_copy(hTo_n[:, :, :],
                                                  ptr1[:, :, 0:BS]),
                            prev_hy1, "cast_c1 after hy_c1")
                        hTo = hTo_n
                        pend = [None, None]

                    for j in range(4):
                        nc.tensor.matmul(g(pbn, j), ones_b[:, 0:BS],
                                         bhn[:, j, :], start=True, stop=False,
                                         tile_position=(0, 32 * j))
                    nrounds(KORD[:4])
                    inject(1)
                    rounds(1, KORD[:4])

                    # ---- odds block: needs hTo (cast_c1 of t-1) ----
                    rounds(0, KORD[4:])          # pb0a stop
                    nrounds(KORD[4:])            # pbn stop
                    rounds(1, KORD[4:])          # pb0b stop

                    # ---- gate chains; engine-queue order pinned:
                    # ACT: sig0 -> sig1 -> tanh0 -> tanh1
                    # DVE: t1_0 -> t2_0 -> t1_1 -> t2_1 -> v2_0 -> hy_0
                    #      -> cast_c0 -> v2_1 -> hy_1 (cast_c1 next iter)
                    ri0 = gwork.tile([128, 2 * C], b16, tag="ri0")
                    sig0 = nc.scalar.activation(ri0[0:P4, :], pb0a[0:P4, :],
                                                AF.Sigmoid)
                    ri1 = gwork.tile([128, 2 * C], b16, tag="ri1")
                    sig1 = after(nc.scalar.activation(ri1[0:P4, :],
                                                      pb0b[0:P4, :], AF.Sigmoid),
                                 sig0, "sig1 after sig0")
                    ric = [ri0, ri1]

                    dve_prev = cast_c1
                    t2s, ngs, v3s = [], [], []
                    for c in range(2):
                        t1 = gwork.tile([128, C], b16, tag=f"t1{c}")
                        dve_prev = after(
                            nc.vector.tensor_mul(t1[0:P4, :], ric[c][0:P4, 0:C],
                                                 pbn[0:P4, C * c:C * (c + 1)]),
                            dve_prev, "dve order")
                        t2 = gwork.tile([128, C], b16, tag=f"t2{c}")
                        dve_prev = after(
                            nc.vector.tensor_add(
                                t2[0:P4, :], t1[0:P4, :],
                                gxt[0:P4, 2 * Q + C * c:2 * Q + C * (c + 1)]),
                            dve_prev, "dve order")
                        t2s.append(t2)
                    act_prev = sig1
                    for c in range(2):
                        ng = gwork.tile([128, C], b16, tag=f"ng{c}")
                        act_prev = after(
                            nc.scalar.activation(ng[0:P4, :], t2s[c][0:P4, :],
                                                 AF.Tanh),
                            act_prev, "tanh order")
                        ngs.append(ng)
                        # v3 = i * h_prev  (GpSimd, off critical path)
                        v3 = gwork.tile([128, C], b16, tag=f"v3{c}")
                        nc.gpsimd.tensor_mul(v3[0:P4, :], ric[c][0:P4, C:2 * C],
                                             hprev[0:P4, C * c:C * (c + 1)])
                        v3s.append(v3)

                    def upd(c, prev):
                        # v2' = (i - 1) * n  (fused);  h' = v3 - v2'
                        v2 = gwork.tile([128, C], b16, tag=f"v2{c}")
                        prev = after(
                            nc.vector.scalar_tensor_tensor(
                                v2[0:P4, :], ric[c][0:P4, C:2 * C], 1.0,
                                ngs[c][0:P4, :], op0=ALU.subtract, op1=ALU.mult),
                            prev, "dve order")
                        return after(
                            nc.vector.tensor_sub(
                                hnew[0:P4, C * c:C * (c + 1)],
                                v3s[c][0:P4, :], v2[0:P4, :]),
                            prev, "dve order")

                    dve_prev = upd(0, dve_prev)

                    # transpose chunk 0 immediately; its cast runs on the
                    # Scalar engine (idle after tanh1) so DVE can finish the
                    # chunk-1 chain without queuing behind it
                    junk(6)
                    ptr0 = ptrpool.tile([128, 4, 32], b16, tag="pt")
                    nc.tensor.transpose(ptr0[:, :, :], hnew[0:P4, 0:C],
                                        identTb[0:P4, :])
                    hTe_n = htepool.tile([128, 4, BS], b16, tag="hTe")
                    act_prev = after(
                        nc.scalar.activation(hTe_n[:, :, :], ptr0[:, :, 0:BS],
                                             AF.Copy),
                        act_prev, "cast_c0 on ACT after tanh1")
                    hTe = hTe_n

                    hy1 = upd(1, dve_prev)
                    junk(1)
                    pend = [hnew, hy1]
                    hprev = hnew

                # flush the last deferred transpose
                if pend[0] is not None:
                    hnp, prev_hy1 = pend
                    ptr1 = ptrpool.tile([128, 4, 32], b16, tag="pt")
                    nc.tensor.transpose(ptr1[:, :, :], hnp[0:P4, C:2 * C],
                                        identTb[0:P4, :])
                    hTo_n = htopool.tile([128, 4, BS], b16, tag="hTo")
                    nc.vector.tensor_copy(hTo_n[:, :, :], ptr1[:, :, 0:BS])
                    hTo = hTo_n

                # ---------- final FC ----------
                po = ptrpool.tile([BS, OUT], f32, tag="pt")
                for k in range(8):
                    nc.tensor.matmul(po[:], hT_blk(k), wfcT[:, k, :],
                                     start=(k == 0), stop=False)
                nc.tensor.matmul(po[:], ones_f[:], bfc[:],
                                 start=False, stop=True)
                ob = gwork.tile([BS, OUT], f32, tag="ri0")
                nc.vector.tensor_copy(ob[:], po[:])
                nc.sync.dma_start(out_d[:], ob[:])

    nc.compile()
    return nc


def _prep_inputs(x, Wx, bx, Wh, bh, Wfc, bfc):
    """Host-side layout prep -> list of per-core input dicts."""
    import ml_dtypes
    bf16 = ml_dtypes.bfloat16

    x = np.asarray(x, np.float32)
    Wx = np.asarray(Wx, np.float32)
    bx = np.asarray(bx, np.float32)
    Wh = np.asarray(Wh, np.float32)
    bh = np.asarray(bh, np.float32)
    Wfc = np.asarray(Wfc, np.float32)
    bfc = np.asarray(bfc, np.float32)

    # grouped gate-column order per quarter j: [r0 | i0 | r1 | i1 | n(256)]
    # (rc/ic are 128-chunks of the quarter)
    blocks = []
    for j in range(4):
        base = j * Q
        for c in range(2):
            blocks.append(0 * H + base + c * C + np.arange(C))  # r chunk
            blocks.append(1 * H + base + c * C + np.arange(C))  # i chunk
        blocks.append(2 * H + base + np.arange(Q))              # n
    perm = np.concatenate(blocks)                 # [3072] grouped row index

    WxT_g = np.ascontiguousarray(Wx[perm].T)      # [256, 3072-grouped]
    bias_ri = bx + np.concatenate([bh[:2 * H], np.zeros(H, np.float32)])
    biasg = bias_ri[perm].reshape(1, H3)

    WhT = Wh.T                                     # [1024 hid, 3072 gates]
    whri = np.empty((8, 128, 4, 2, 2 * C), np.float32)
    whn = np.empty((8, 128, 4, Q), np.float32)
    for k in range(8):
        hid = slice(k * 128, (k + 1) * 128)
        for j in range(4):
            for c in range(2):
                q = slice(j * Q + c * C, j * Q + c * C + C)
                whri[k, :, j, c, :C] = WhT[hid, 0 * H:1 * H][:, q]
                whri[k, :, j, c, C:] = WhT[hid, 1 * H:2 * H][:, q]
            whn[k, :, j, :] = WhT[hid, 2 * H:3 * H][:, j * Q:(j + 1) * Q]
    bhn = bh[2 * H:].reshape(4, Q)[None, :, :]

    wfcT = np.ascontiguousarray(Wfc.T).reshape(8, 128, OUT)
    bfc2 = bfc.reshape(1, OUT)

    # transpose identity: diag 1 at columns where (col % 32) < 8, rows 0:104
    identT = np.zeros((128, 128), np.float32)
    for p in range(P4):
        if p % 32 < BS:
            identT[p, p] = 1.0

    common = {
        "wxg": WxT_g.reshape(2, 128, H3).astype(bf16),
        "biasg": biasg.astype(bf16),
        "whri": whri.astype(bf16),
        "whn": whn.astype(bf16),
        "bhn": bhn.astype(bf16),
        "wfcT": wfcT.astype(bf16),
        "bfc": bfc2,
        "identg": np.tile(np.vstack([np.eye(8, dtype=np.float32),
                                     np.zeros((24, 8), np.float32)]),
                          (4, 1)).astype(bf16),
        "identT": identT,
    }

    in_maps = []
    for cc in range(NCORES):
        xs = x[cc * BS:(cc + 1) * BS]             # [BS, T, IN]
        xsT = xs.transpose(2, 1, 0)               # [IN, T, BS]
        in_maps.append({
            "xsT": np.ascontiguousarray(xsT.reshape(2, 128, T * BS)).astype(bf16),
            **common,
        })
    return in_maps


def kernel(x, Wx, bx, Wh, bh, Wfc, bfc):
    from concourse.bass_utils import run_bass_kernel_spmd

    if "nc" not in _cache:
        _cache["nc"] = _build_program()
    nc = _cache["nc"]

    in_maps = _prep_inputs(x, Wx, bx, Wh, bh, Wfc, bfc)
    res = run_bass_kernel_spmd(nc, in_maps, list(range(NCORES)))
    out = np.concatenate([res.results[c]["out"] for c in range(NCORES)], axis=0)
    return out.astype(np.float32)


if __name__ == "__main__":
    rng = np.random.default_rng(0)
    std = 1.0 / np.sqrt(H)
    inputs = {
        "x": rng.standard_normal((B, T, IN), dtype=np.float32),
        "Wx": rng.uniform(-std, std, (H3, IN)).astype(np.float32),
        "bx": rng.uniform(-std, std, (H3,)).astype(np.float32),
        "Wh": rng.uniform(-std, std, (H3, H)).astype(np.float32),
        "bh": rng.uniform(-std, std, (H3,)).astype(np.float32),
        "Wfc": rng.uniform(-std, std, (OUT, H)).astype(np.float32),
        "bfc": rng.uniform(-std, std, (OUT,)).astype(np.float32),
    }
    out = kernel(**inputs)
    print("out", out.shape, out.dtype)
    print(out[:2])


# revision 27
# speedup vs baseline: 1.1944x; 1.0708x over previous
"""GRU kernel for Trainium2 (8 NeuronCores, SPMD).

Problem: B=64, T=512, IN=256, H=1024, OUT=10
  gate_x_all = x @ Wx.T + bx            [B,T,3H]
  scan over T:  gate_h = h @ Wh.T + bh
                r = sig(i_r + h_r); i = sig(i_i + h_i)
                n = tanh(i_n + r * h_n)
                h = n + i * (h - n)
  out = h_last @ Wfc.T + bfc            [B,OUT]

Strategy (v10): data-parallel over batch (8 rows/core), weights replicated.
 - bf16 operands everywhere (incl. the h state), fp32 PSUM accumulation.
 - Phase 1: gate_x precomputed for all T in one large weight-streaming GEMM.
 - Phase 2: the T-step scan, software-pipelined in 2 feature-chunks:
   * gate columns per hidden-quarter j ordered [r0|i0|r1|i1|n] (128-chunks)
     so ONE sigmoid covers a chunk's r and i halves.
   * per-step PSUM: pbn (n-gate) + pb0c0/pb0c1 (r,i per chunk) with separate
     accumulation groups -> chunk-0's sigmoid/tanh chain runs while chunk-1
     matmuls still stream.
   * h update: i=sig(.); v2'=(i-1)*tanh-gate (fused); v3=i*h (GpSimd);
     h' = v3 - v2'.
   * h' -> hT via 2 full-width PE transposes (one per 128-chunk), cast into
     separate even/odd hT tiles; next step's matmul k-rounds run evens first
     so streaming restarts before the second chunk's chain finishes.
"""

import os
import sys

import numpy as np

for p in ("/root/.axon_site", "/root/.axon_site/_ro/trn_rl_repo",
          "/root/.axon_site/_ro/pypackages", "/opt/trn_rl_repo"):
    if p not in sys.path:
        sys.path.append(p)

B, T, IN, H, OUT = 64, 512, 256, 1024, 10
H3 = 3 * H
NCORES = 8
BS = B // NCORES      # batch per core (8)
Q = H // 4            # hidden quarter (256)
GF = 3 * Q            # per-group gate columns (768): [r0 i0 r1 i1 | n]
C = 128               # feature chunk within a quarter
P4 = 3 * 32 + BS      # 104: partitions covering all 4 batch groups

_cache = {}


def _build_program():
    import concourse.bass as bass
    import concourse.bacc as bacc
    import concourse.mybir as mybir
    from concourse.tile import TileContext
    from concourse.tile_rust import add_dep_helper

    def after(inst, prev, why="order"):
        # pin engine-queue order (sync=False: no semaphore, FIFO suffices)
        if prev is not None:
            add_dep_helper(inst.ins, prev.ins, sync=False, reason=why)
        return inst

    f32 = mybir.dt.float32
    b16 = mybir.dt.bfloat16
    AF = mybir.ActivationFunctionType
    ALU = mybir.AluOpType

    nc = bacc.Bacc(None, target_bir_lowering=False)

    # ---- per-core inputs ----
    xsT_d = nc.dram_tensor("xsT", [2, 128, T * BS], b16, kind="ExternalInput")
    wxg_d = nc.dram_tensor("wxg", [2, 128, H3], b16, kind="ExternalInput")
    biasg_d = nc.dram_tensor("biasg", [1, H3], b16, kind="ExternalInput")
    # Wh^T: r,i interleaved by 128-chunk [8k,128,4j,2c,256(r|i)], n separate
    whri_d = nc.dram_tensor("whri", [8, 128, 4, 2, 2 * C], b16, kind="ExternalInput")
    whn_d = nc.dram_tensor("whn", [8, 128, 4, Q], b16, kind="ExternalInput")
    bhn_d = nc.dram_tensor("bhn", [1, 4, Q], b16, kind="ExternalInput")
    wfcT_d = nc.dram_tensor("wfcT", [8, 128, OUT], b16, kind="ExternalInput")
    bfc_d = nc.dram_tensor("bfc", [1, OUT], f32, kind="ExternalInput")
    ident_d = nc.dram_tensor("identg", [128, 8], b16, kind="ExternalInput")
    identT_d = nc.dram_tensor("identT", [128, 128], f32, kind="ExternalInput")
    out_d = nc.dram_tensor("out", [BS, OUT], f32, kind="ExternalOutput")

    # gate_x for all T: [m=T/16, p=(16t x 8b), 4*GF] in grouped column order
    gx_d = nc.dram_tensor("gx_all", [T // 16, 128, H3], b16)

    t_steps = int(os.environ.get("KERNEL_T", str(T)))
    KORD = (0, 2, 4, 6, 1, 3, 5, 7)   # evens first: hTe ready before hTo

    with TileContext(nc) as tc:
        # ---------- constants ----------
        with tc.tile_pool(name="const", bufs=1) as cpool:
            whri = cpool.tile([128, 8, 4, 2, 2 * C], b16)
            for k in range(8):
                for j in range(4):
                    nc.sync.dma_start(whri[:, k, j, :, :], whri_d[k, :, j, :, :])
            whn = cpool.tile([128, 8, 4, Q], b16)
            for k in range(8):
                for j in range(4):
                    nc.sync.dma_start(whn[:, k, j, :], whn_d[k, :, j, :])
            bhn = cpool.tile([1, 4, Q], b16)
            nc.sync.dma_start(bhn[:], bhn_d[:])
            wfcT = cpool.tile([128, 8, OUT], b16)
            for k in range(8):
                nc.sync.dma_start(wfcT[:, k, :], wfcT_d[k])
            bfc = cpool.tile([1, OUT], f32)
            nc.sync.dma_start(bfc[:], bfc_d[:])
            ones_b = cpool.tile([1, 128], b16)
            nc.gpsimd.memset(ones_b[:], 1.0)
            ones_f = cpool.tile([1, BS], f32)
            nc.gpsimd.memset(ones_f[:], 1.0)
            identg = cpool.tile([128, 8], b16)
            nc.sync.dma_start(identg[:], ident_d[:])
            identT = cpool.tile([128, 128], f32)
            nc.sync.dma_start(identT[:], identT_d[:])
            identTb = cpool.tile([128, 128], b16)
            nc.vector.tensor_copy(identTb[:], identT[:])

            # ---------- phase 1: gate_x precompute ----------
            with (
                tc.tile_pool(name="px", bufs=2) as pxpool,
                tc.tile_pool(name="pxo", bufs=4) as pxopool,
                tc.tile_pool(name="ppre", bufs=3, space="PSUM") as ppre,
            ):
                xsT = pxpool.tile([128, 2, T * BS], b16)
                for k in range(2):
                    for c in range(8):
                        cs = slice(c * 512, (c + 1) * 512)
                        nc.sync.dma_start(xsT[:, k, cs], xsT_d[k, :, cs])
                wxg = pxpool.tile([128, 2, H3], b16)
                for k in range(2):
                    for c in range(6):
                        cs = slice(c * 512, (c + 1) * 512)
                        nc.sync.dma_start(wxg[:, k, cs], wxg_d[k, :, cs])
                biasg = pxpool.tile([1, H3], b16)
                nc.sync.dma_start(biasg[:], biasg_d[:])

                for m in range(32):          # M-tiles: 128 rows = 16 t x 8 b
                    ms = slice(m * 128, (m + 1) * 128)
                    for j in range(4):       # N-chunks of GF=768 (one group)
                        ncs = slice(j * GF, (j + 1) * GF)
                        pp = ppre.tile([128, GF], f32, tag="pp")
                        for k in range(2):
                            for (o, w) in ((0, 512), (512, 256)):
                                nc.tensor.matmul(
                                    pp[:, o:o + w], xsT[:, k, ms],
                                    wxg[:, k, j * GF + o:j * GF + o + w],
                                    start=(k == 0), stop=False)
                        for (o, w) in ((0, 512), (512, 256)):
                            nc.tensor.matmul(
                                pp[:, o:o + w], ones_b[:],
                                biasg[:, j * GF + o:j * GF + o + w],
                                start=False, stop=True)
                        ob = pxopool.tile([128, GF], b16, tag="ob")
                        if j % 2 == 0:
                            nc.vector.tensor_copy(ob[:], pp[:])
                            nc.sync.dma_start(gx_d[m, :, ncs], ob[:])
                        else:
                            nc.scalar.activation(ob[:], pp[:], AF.Copy)
                            nc.scalar.dma_start(gx_d[m, :, ncs], ob[:])

            # ---------- phase 2: the scan ----------
            with (
                tc.tile_pool(name="state", bufs=3) as spool,
                tc.tile_pool(name="hte", bufs=2) as htepool,
                tc.tile_pool(name="hto", bufs=2) as htopool,
                tc.tile_pool(name="gxt", bufs=6) as gxpool,
                tc.tile_pool(name="gw", bufs=3) as gwork,
                tc.tile_pool(name="pbn", bufs=1, space="PSUM") as pbnpool,
                tc.tile_pool(name="pba", bufs=2, space="PSUM") as pbapool,
                tc.tile_pool(name="pbb", bufs=2, space="PSUM") as pbbpool,
                tc.tile_pool(name="ptr", bufs=2, space="PSUM") as ptrpool,
                tc.tile_pool(name="pwm", bufs=1, space="PSUM") as pwmpool,
            ):
                # HAM keep-warm target: junk matmuls during chain waits keep
                # the PE clock at 8/8 (sub-us idle gaps otherwise oscillate it)
                pwarm = pwmpool.tile([128, 512], f32, tag="pw")
                hTe = htepool.tile([128, 4, BS], b16, tag="hTe")
                nc.gpsimd.memset(hTe[:], 0.0)
                hTo = htopool.tile([128, 4, BS], b16, tag="hTo")
                nc.gpsimd.memset(hTo[:], 0.0)
                hprev = spool.tile([128, Q], b16, tag="h")
                nc.gpsimd.memset(hprev[:], 0.0)

                def g(ap, j):
                    return ap[32 * j:32 * j + BS]

                def hT_blk(k):
                    src = hTe if k % 2 == 0 else hTo
                    return src[:, k // 2, :]

                def junk(n):
                    # deps-free PE filler (constant lhsT, constant rhs)
                    for w in range(n):
                        nc.tensor.matmul(pwarm[0:BS, :], identg[:, 0:BS],
                                         whri[:, w % 8, 0, :, :],
                                         start=True, stop=True)

                # previous step's hnew (chunk-1 transpose deferred into the
                # next step's evens block) + its last DVE chain op
                pend = [None, None]   # (hnew_tile, hy_c1_inst)

                for t in range(t_steps):
                    m, tt = t // 16, t % 16
                    gxt = gxpool.tile([128, GF], b16, tag="gxt")
                    for j in range(4):
                        nc.sync.dma_start(
                            gxt[32 * j:32 * j + BS, :],
                            gx_d[m, tt * BS:(tt + 1) * BS,
                                 j * GF:(j + 1) * GF])

                    pbn = pbnpool.tile([128, Q], f32, tag="pbn")
                    pb0a = pbapool.tile([128, 2 * C], f32, tag="pa")
                    pb0b = pbbpool.tile([128, 2 * C], f32, tag="pb")
                    pb0 = [pb0a, pb0b]
                    hnew = spool.tile([128, Q], b16, tag="h")

                    def inject(c):
                        for j in range(4):
                            nc.tensor.matmul(
                                g(pb0[c], j), g(identg, j),
                                gxt[32 * j:32 * j + BS, 2 * C * c:2 * C * (c + 1)],
                                start=True, stop=False,
                                tile_position=(32 * j, 32 * j))

                    def rounds(c, ks, pin=None):
                        for k in ks:
                            for j in range(4):
                                inst = nc.tensor.matmul(
                                    g(pb0[c], j), hT_blk(k),
                                    whri[:, k, j, c, :],
                                    start=False, stop=(k == 7),
                                    tile_position=(0, 32 * j))
                                if pin == "start":
                                    pin = inst
                                elif pin is not None:
                                    pin = after(inst, pin, "pe odds order")
                        return pin

                    def nrounds(ks, pin=None):
                        for k in ks:
                            for j in range(4):
                                inst = nc.tensor.matmul(
                                    g(pbn, j), hT_blk(k),
                                    whn[:, k, j, :],
                                    start=False, stop=(k == 7),
                                    tile_position=(0, 32 * j))
                                if pin is not None:
                                    pin = after(inst, pin, "pe odds order")
                        return pin

                    # ---- evens block: needs only hTe (cast_c0 of t-1) ----
                    junk(2)
                    inject(0)
                    rounds(0, KORD[:4])
                    for j in range(4):
                        nc.tensor.matmul(g(pbn, j), ones_b[:, 0:BS],
                                         bhn[:, j, :], start=True, stop=False,
                                         tile_position=(0, 32 * j))
                    nrounds(KORD[:4])

                    # deferred: transpose prev step's hnew chunk 1 -> hTo
                    cast_c1 = None
                    if pend[0] is not None:
                        hnp, prev_hy1 = pend
                        ptr1 = ptrpool.tile([128, 4, 32], b16, tag="pt")
                        nc.tensor.transpose(ptr1[:, :, :], hnp[0:P4, C:2 * C],
                                            identTb[0:P4, :])
                        hTo_n = htopool.tile([128, 4, BS], b16, tag="hTo")
                        cast_c1 = after(
                            nc.vector.tensor_copy(hTo_n[:, :, :],
                                                  ptr1[:, :, 0:BS]),
                            prev_hy1, "cast_c1 after hy_c1")
                        hTo = hTo_n
                        pend = [None, None]

                    inject(1)
                    rounds(1, KORD[:4])

                    # ---- odds block: needs hTo (cast_c1 of t-1) ----
                    # PE order pinned so ri_c0's stop (which releases sig0,
                    # the longest chain) always lands first
                    pe = rounds(0, KORD[4:], pin="start")  # pb0a stop
                    pe = nrounds(KORD[4:], pin=pe)           # pbn stop
                    pe = rounds(1, KORD[4:], pin=pe)         # pb0b stop

                    # ---- gate chains; engine-queue order pinned:
                    # ACT: sig0 -> sig1 -> tanh0 -> tanh1
                    # DVE: t1_0 -> t2_0 -> t1_1 -> t2_1 -> v2_0 -> hy_0
                    #      -> cast_c0 -> v2_1 -> hy_1 (cast_c1 next iter)
                    ri0 = gwork.tile([128, 2 * C], b16, tag="ri0")
                    sig0 = nc.scalar.activation(ri0[0:P4, :], pb0a[0:P4, :],
                                                AF.Sigmoid)
                    ri1 = gwork.tile([128, 2 * C], b16, tag="ri1")
                    sig1 = after(nc.scalar.activation(ri1[0:P4, :],
                                                      pb0b[0:P4, :], AF.Sigmoid),
                                 sig0, "sig1 after sig0")
                    ric = [ri0, ri1]

                    dve_prev = cast_c1
                    t2s, ngs, v3s = [], [], []
                    for c in range(2):
                        t1 = gwork.tile([128, C], b16, tag=f"t1{c}")
                        dve_prev = after(
                            nc.vector.tensor_mul(t1[0:P4, :], ric[c][0:P4, 0:C],
                                                 pbn[0:P4, C * c:C * (c + 1)]),
                            dve_prev, "dve order")
                        t2 = gwork.tile([128, C], b16, tag=f"t2{c}")
                        dve_prev = after(
                            nc.vector.tensor_add(
                                t2[0:P4, :], t1[0:P4, :],
                                gxt[0:P4, 2 * Q + C * c:2 * Q + C * (c + 1)]),
                            dve_prev, "dve order")
                        t2s.append(t2)
                    act_prev = sig1
                    for c in range(2):
                        ng = gwork.tile([128, C], b16, tag=f"ng{c}")
                        act_prev = after(
                            nc.scalar.activation(ng[0:P4, :], t2s[c][0:P4, :],
                                                 AF.Tanh),
                            act_prev, "tanh order")
                        ngs.append(ng)
                        # v3 = i * h_prev  (GpSimd, off critical path)
                        v3 = gwork.tile([128, C], b16, tag=f"v3{c}")
                        nc.gpsimd.tensor_mul(v3[0:P4, :], ric[c][0:P4, C:2 * C],
                                             hprev[0:P4, C * c:C * (c + 1)])
                        v3s.append(v3)

                    def upd(c, prev):
                        # v2' = (i - 1) * n  (fused);  h' = v3 - v2'
                        v2 = gwork.tile([128, C], b16, tag=f"v2{c}")
                        prev = after(
                            nc.vector.scalar_tensor_tensor(
                                v2[0:P4, :], ric[c][0:P4, C:2 * C], 1.0,
                                ngs[c][0:P4, :], op0=ALU.subtract, op1=ALU.mult),
                            prev, "dve order")
                        return after(
                            nc.vector.tensor_sub(
                                hnew[0:P4, C * c:C * (c + 1)],
                                v3s[c][0:P4, :], v2[0:P4, :]),
                            prev, "dve order")

                    dve_prev = upd(0, dve_prev)

                    # transpose + cast chunk 0 immediately (gates t+1 evens)
                    junk(3)
                    ptr0 = ptrpool.tile([128, 4, 32], b16, tag="pt")
                    nc.tensor.transpose(ptr0[:, :, :], hnew[0:P4, 0:C],
                                        identTb[0:P4, :])
                    hTe_n = htepool.tile([128, 4, BS], b16, tag="hTe")
                    dve_prev = after(
                        nc.vector.tensor_copy(hTe_n[:, :, :], ptr0[:, :, 0:BS]),
                        dve_prev, "cast_c0 after hy_c0")
                    hTe = hTe_n

                    hy1 = upd(1, dve_prev)
                    junk(1)
                    pend = [hnew, hy1]
                    hprev = hnew

                # flush the last deferred transpose
                if pend[0] is not None:
                    hnp, prev_hy1 = pend
                    ptr1 = ptrpool.tile([128, 4, 32], b16, tag="pt")
                    nc.tensor.transpose(ptr1[:, :, :], hnp[0:P4, C:2 * C],
                                        identTb[0:P4, :])
                    hTo_n = htopool.tile([128, 4, BS], b16, tag="hTo")
                    nc.vector.tensor_copy(hTo_n[:, :, :], ptr1[:, :, 0:BS])
                    hTo = hTo_n

                # ---------- final FC ----------
                po = ptrpool.tile([BS, OUT], f32, tag="pt")
                for k in range(8):
                    nc.tensor.matmul(po[:], hT_blk(k), wfcT[:, k, :],
                                     start=(k == 0), stop=False)
                nc.tensor.matmul(po[:], ones_f[:], bfc[:],
                                 start=False, stop=True)
                ob = gwork.tile([BS, OUT], f32, tag="ri0")
                nc.vector.tensor_copy(ob[:], po[:])
                nc.sync.dma_start(out_d[:], ob[:])

    nc.compile()
    return nc


def _prep_inputs(x, Wx, bx, Wh, bh, Wfc, bfc):
    """Host-side layout prep -> list of per-core input dicts."""
    import ml_dtypes
    bf16 = ml_dtypes.bfloat16

    x = np.asarray(x, np.float32)
    Wx = np.asarray(Wx, np.float32)
    bx = np.asarray(bx, np.float32)
    Wh = np.asarray(Wh, np.float32)
    bh = np.asarray(bh, np.float32)
    Wfc = np.asarray(Wfc, np.float32)
    bfc = np.asarray(bfc, np.float32)

    # grouped gate-column order per quarter j: [r0 | i0 | r1 | i1 | n(256)]
    # (rc/ic are 128-chunks of the quarter)
    blocks = []
    for j in range(4):
        base = j * Q
        for c in range(2):
            blocks.append(0 * H + base + c * C + np.arange(C))  # r chunk
            blocks.append(1 * H + base + c * C + np.arange(C))  # i chunk
        blocks.append(2 * H + base + np.arange(Q))              # n
    perm = np.concatenate(blocks)                 # [3072] grouped row index

    WxT_g = np.ascontiguousarray(Wx[perm].T)      # [256, 3072-grouped]
    bias_ri = bx + np.concatenate([bh[:2 * H], np.zeros(H, np.float32)])
    biasg = bias_ri[perm].reshape(1, H3)

    WhT = Wh.T                                     # [1024 hid, 3072 gates]
    whri = np.empty((8, 128, 4, 2, 2 * C), np.float32)
    whn = np.empty((8, 128, 4, Q), np.float32)
    for k in range(8):
        hid = slice(k * 128, (k + 1) * 128)
        for j in range(4):
            for c in range(2):
                q = slice(j * Q + c * C, j * Q + c * C + C)
                whri[k, :, j, c, :C] = WhT[hid, 0 * H:1 * H][:, q]
                whri[k, :, j, c, C:] = WhT[hid, 1 * H:2 * H][:, q]
            whn[k, :, j, :] = WhT[hid, 2 * H:3 * H][:, j * Q:(j + 1) * Q]
    bhn = bh[2 * H:].reshape(4, Q)[None, :, :]

    wfcT = np.ascontiguousarray(Wfc.T).reshape(8, 128, OUT)
    bfc2 = bfc.reshape(1, OUT)

    # transpose identity: diag 1 at columns where (col % 32) < 8, rows 0:104
    identT = np.zeros((128, 128), np.float32)
    for p in range(P4):
        if p % 32 < BS:
            identT[p, p] = 1.0

    common = {
        "wxg": WxT_g.reshape(2, 128, H3).astype(bf16),
        "biasg": biasg.astype(bf16),
        "whri": whri.astype(bf16),
        "whn": whn.astype(bf16),
        "bhn": bhn.astype(bf16),
        "wfcT": wfcT.astype(bf16),
        "bfc": bfc2,
        "identg": np.tile(np.vstack([np.eye(8, dtype=np.float32),
                                     np.zeros((24, 8), np.float32)]),
                          (4, 1)).astype(bf16),
        "identT": identT,
    }

    in_maps = []
    for cc in range(NCORES):
        xs = x[cc * BS:(cc + 1) * BS]             # [BS, T, IN]
        xsT = xs.transpose(2, 1, 0)               # [IN, T, BS]
        in_maps.append({
            "xsT": np.ascontiguousarray(xsT.reshape(2, 128, T * BS)).astype(bf16),
            **common,
        })
    return in_maps


def kernel(x, Wx, bx, Wh, bh, Wfc, bfc):
    from concourse.bass_utils import run_bass_kernel_spmd

    if "nc" not in _cache:
        _cache["nc"] = _build_program()
    nc = _cache["nc"]

    in_maps = _prep_inputs(x, Wx, bx, Wh, bh, Wfc, bfc)
    res = run_bass_kernel_spmd(nc, in_maps, list(range(NCORES)))
    out = np.concatenate([res.results[c]["out"] for c in range(NCORES)], axis=0)
    return out.astype(np.float32)


if __name__ == "__main__":
    rng = np.random.default_rng(0)
    std = 1.0 / np.sqrt(H)
    inputs = {
        "x": rng.standard_normal((B, T, IN), dtype=np.float32),
        "Wx": rng.uniform(-std, std, (H3, IN)).astype(np.float32),
        "bx": rng.uniform(-std, std, (H3,)).astype(np.float32),
        "Wh": rng.uniform(-std, std, (H3, H)).astype(np.float32),
        "bh": rng.uniform(-std, std, (H3,)).astype(np.float32),
        "Wfc": rng.uniform(-std, std, (OUT, H)).astype(np.float32),
        "bfc": rng.uniform(-std, std, (OUT,)).astype(np.float32),
    }
    out = kernel(**inputs)
    print("out", out.shape, out.dtype)
    print(out[:2])
